# revision 1
# baseline (speedup 1.0000x reference)
"""Trainium2 Bass kernel for nn_CrossAttention_82471962018390.

Dilated (d=2) 9x9 neighborhood cross-attention, q 48x48 vs k/v 24x24.

Math identity used: the nearest-exact 2x upsample + dilation-2 NATTEN window
collapses so that query (h, w) attends to the ORIGINAL 24x24 k/v grid at
rows clip(h//2-4, 0, 15) + 0..8, cols clip(w//2-4, 0, 15) + 0..8 (a
contiguous 9x9 window; the 4 queries in each 2x2 block share one window).

Kernel structure (per (b, head) pair; 2 pairs per core, 8 cores = 16 pairs):
  - 16 row-bands by s_h = clip(h//2-4,0,15); band s attends the 9x24=216-key
    slab k[:, s:s+9, :].
  - Scores computed transposed: S^T[key, query] = (K slab)^T @ Q, with the
    column-window mask folded INTO the matmul via 16 extra contraction rows:
    lhsT rows 64:80 hold M0[r, kw] (0 or -30), rhs rows 64:80 hold the
    one-hot of s_w(w).  scale 1/8 is folded into q on the host.
  - exp on ScalarE (PSUM -> SBUF bf16).
  - One PV matmul per key-chunk with stationary [V^T | ones*64] giving
    [PV (64 rows); sumexp replicated (64 rows)] in one PSUM tile.
  - One VectorE divide -> output tile, DMA out.
Bands are grouped (10/2*5/2*5/2*4/10 h-rows) so ACT/DVE instructions run on
~480-column tiles.
"""

import numpy as np
import ml_dtypes

try:
    import concourse.bass as bass
    import concourse.bacc as bacc
    import concourse.tile as tile
    from concourse import mybir
    from concourse.bass_utils import run_bass_kernel_spmd
except ImportError:  # pragma: no cover
    import sys

    sys.path.insert(0, "/opt/trn_rl_repo")
    import concourse.bass as bass
    import concourse.bacc as bacc
    import concourse.tile as tile
    from concourse import mybir
    from concourse.bass_utils import run_bass_kernel_spmd

from contextlib import ExitStack

BF = ml_dtypes.bfloat16
N_CORES = 8
NPAIR = 2  # (b, head) pairs per core
DH = 64
HQ = WQ = 48
HK = WK = 24
NQ = HQ * WQ  # 2304
NK = HK * WK  # 576
BAND_KEYS = 9 * WK  # 216
CH1 = 128  # keys in chunk 1 of a band
CH2 = BAND_KEYS - CH1  # 88

# s(i) = clip(i//2 - 4, 0, 15) for i in 0..47
_S = np.clip(np.arange(48) // 2 - 4, 0, 15)

# Band groups: (q column offset, width, [(band s, rel q offset, band width)])
# band s covers h rows where s_h(h) == s: s=0 -> h 0..9, s=1..14 -> 2 rows,
# s=15 -> h 38..47.
def _groups():
    bands_h0 = {}
    for h in range(48):
        bands_h0.setdefault(int(_S[h]), []).append(h)
    spans = {s: (hs[0], len(hs)) for s, hs in bands_h0.items()}
    layout = [[0], [1, 2, 3, 4, 5], [6, 7, 8, 9, 10], [11, 12, 13, 14], [15]]
    groups = []
    for g in layout:
        h0 = spans[g[0]][0]
        width = sum(spans[s][1] for s in g) * 48
        bands = []
        off = 0
        for s in g:
            bw = spans[s][1] * 48
            bands.append((s, off, bw))
            off += bw
        groups.append((h0 * 48, width, bands))
    return groups


GROUPS = _groups()
MAXM = max(w for _, w, _ in GROUPS)  # 480


def _host_tables():
    """M0exp [16, 576] and Bw [16, 2304] mask/one-hot tables (fp32)."""
    m0 = np.full((16, WK), -30.0, np.float32)
    for r in range(16):
        m0[r, r : r + 9] = 0.0
    m0exp = np.tile(m0[:, None, :], (1, HK, 1)).reshape(16, NK)
    bw = np.zeros((16, NQ), np.float32)
    for w in range(48):
        bw[_S[w], np.arange(48) * 48 + w] = 1.0
    return m0exp, bw


def build_kernel(ctx: ExitStack, tc, qb, km, vb, out):
    nc = tc.nc
    FP32 = mybir.dt.float32
    BF16 = mybir.dt.bfloat16
    Exp = mybir.ActivationFunctionType.Exp
    div = mybir.AluOpType.divide

    qpool = ctx.enter_context(tc.tile_pool(name="qb", bufs=2))
    kpool = ctx.enter_context(tc.tile_pool(name="km", bufs=2))
    vpool = ctx.enter_context(tc.tile_pool(name="vt", bufs=1))
    spool = ctx.enter_context(tc.tile_pool(name="scores", bufs=2, space="PSUM"))
    opool = ctx.enter_context(tc.tile_pool(name="opsum", bufs=2, space="PSUM"))
    epool = ctx.enter_context(tc.tile_pool(name="expo", bufs=2))
    rpool = ctx.enter_context(tc.tile_pool(name="res", bufs=2))

    # Rotating persistent V'' stationary tiles: cols 0:64 = V^T chunk (DMA'd
    # per band), cols 64:128 = ones (memset once; gives replicated sumexp).
    NVT = 8
    vtiles = [vpool.tile([128, 128], BF16, tag=f"vt{j}", name=f"vt{j}") for j in range(NVT)]
    for vt in vtiles:
        nc.vector.memset(vt[:, 64:128], 1.0)
    vcount = 0

    for p in range(NPAIR):
        qb_t = qpool.tile([80, NQ], BF16)
        nc.sync.dma_start(qb_t[:, :], qb[80 * p : 80 * p + 80, :])
        km_t = kpool.tile([80, NK], BF16)
        nc.sync.dma_start(km_t[:, :], km[80 * p : 80 * p + 80, :])

        for (q0, M, bands) in GROUPS:
            s1 = spool.tile([128, MAXM], FP32, tag="s1")
            s2 = spool.tile([128, MAXM], FP32, tag="s2")
            for (s, off, bw) in bands:
                rhs = qb_t[:, q0 + off : q0 + off + bw]
                nc.tensor.matmul(
                    s1[:, off : off + bw],
                    km_t[:, 24 * s : 24 * s + CH1],
                    rhs,
                    start=True,
                    stop=True,
                )
                nc.tensor.matmul(
                    s2[0:CH2, off : off + bw],
                    km_t[:, 24 * s + CH1 : 24 * s + BAND_KEYS],
                    rhs,
                    start=True,
                    stop=True,
                )
            e1 = epool.tile([128, MAXM], BF16, tag="e1")
            e2 = epool.tile([128, MAXM], BF16, tag="e2")
            nc.scalar.activation(e1[:, :M], s1[:, :M], Exp)
            nc.scalar.activation(e2[0:CH2, :M], s2[0:CH2, :M], Exp)

            o = opool.tile([128, MAXM], FP32, tag="o")
            for (s, off, bw) in bands:
                vta = vtiles[vcount % NVT]
                vtb = vtiles[(vcount + 1) % NVT]
                vcount += 2
                row0 = (p * 16 + s) * BAND_KEYS
                nc.sync.dma_start(vta[:, 0:64], vb[row0 : row0 + CH1, :])
                nc.sync.dma_start(vtb[0:CH2, 0:64], vb[row0 + CH1 : row0 + BAND_KEYS, :])
                # Same-output WAW dep keeps the accumulation pair ordered;
                # CoreSim's psum-group check validates the final schedule.
                nc.tensor.matmul(
                    o[:, off : off + bw],
                    vta[:, :],
                    e1[:, off : off + bw],
                    start=True,
                    stop=False,
                )
                nc.tensor.matmul(
                    o[:, off : off + bw],
                    vtb[0:CH2, :],
                    e2[0:CH2, off : off + bw],
                    start=False,
                    stop=True,
                )
            # DVE can read only ONE PSUM operand per instruction: reciprocal
            # of the replicated sumexp rows PSUM->SBUF, then PV * recip.
            rcp = rpool.tile([64, MAXM], FP32, tag="rcp")
            nc.vector.reciprocal(rcp[:, :M], o[64:128, :M])
            res = rpool.tile([64, MAXM], FP32, tag="res")
            nc.vector.tensor_mul(res[:, :M], o[0:64, :M], rcp[:, :M])
            nc.sync.dma_start(out[64 * p : 64 * p + 64, q0 : q0 + M], res[:, :M])


_CACHE = {}


def _get_nc():
    if "nc" not in _CACHE:
        nc = bacc.Bacc(
            "TRN2", target_bir_lowering=False, debug=False, num_devices=N_CORES
        )
        qb = nc.dram_tensor(
            "qb", [NPAIR * 80, NQ], mybir.dt.bfloat16, kind="ExternalInput"
        ).ap()
        km = nc.dram_tensor(
            "km", [NPAIR * 80, NK], mybir.dt.bfloat16, kind="ExternalInput"
        ).ap()
        vb = nc.dram_tensor(
            "vb", [NPAIR * 16 * BAND_KEYS, DH], mybir.dt.bfloat16, kind="ExternalInput"
        ).ap()
        out = nc.dram_tensor(
            "out", [NPAIR * 64, NQ], mybir.dt.float32, kind="ExternalOutput"
        ).ap()
        with tile.TileContext(nc) as tc, ExitStack() as ctx:
            build_kernel(ctx, tc, qb, km, vb, out)
        nc.compile()
        _CACHE["nc"] = nc
    return _CACHE["nc"]


def kernel(q: np.ndarray, k: np.ndarray, v: np.ndarray) -> np.ndarray:
    assert q.shape == (2, 512, HQ, WQ) and k.shape == (2, 512, HK, WK)
    m0exp, bw = _host_tables()
    nc = _get_nc()

    in_maps = []
    for c in range(N_CORES):
        qbc = np.empty((NPAIR * 80, NQ), BF)
        kmc = np.empty((NPAIR * 80, NK), BF)
        vbc = np.empty((NPAIR * 16 * BAND_KEYS, DH), BF)
        for pl in range(NPAIR):
            pg = NPAIR * c + pl
            b, hd = pg // 8, pg % 8
            qbc[80 * pl : 80 * pl + 64] = (
                q[b, 64 * hd : 64 * hd + 64].reshape(64, NQ) / 8.0
            ).astype(BF)
            qbc[80 * pl + 64 : 80 * pl + 80] = bw.astype(BF)
            kmc[80 * pl : 80 * pl + 64] = (
                k[b, 64 * hd : 64 * hd + 64].reshape(64, NK).astype(BF)
            )
            kmc[80 * pl + 64 : 80 * pl + 80] = m0exp.astype(BF)
            v3 = v[b, 64 * hd : 64 * hd + 64].reshape(64, HK, WK)
            for s in range(16):
                row0 = (pl * 16 + s) * BAND_KEYS
                vbc[row0 : row0 + BAND_KEYS] = (
                    v3[:, s : s + 9, :].reshape(64, BAND_KEYS).T.astype(BF)
                )
        in_maps.append({"qb": qbc, "km": kmc, "vb": vbc})

    results = run_bass_kernel_spmd(nc, in_maps, list(range(N_CORES))).results

    out = np.empty((2, 512, HQ, WQ), np.float32)
    for c in range(N_CORES):
        o = results[c]["out"]
        for pl in range(NPAIR):
            pg = NPAIR * c + pl
            b, hd = pg // 8, pg % 8
            out[b, 64 * hd : 64 * hd + 64] = o[64 * pl : 64 * pl + 64].reshape(
                64, HQ, WQ
            )
    return out


if __name__ == "__main__":
    qq = np.load("/root/problem/q.npy")
    kk = np.load("/root/problem/k.npy")
    vv = np.load("/root/problem/v.npy")
    got = kernel(qq, kk, vv)
    exp = np.load("/root/problem/expected.npy")
    rel = np.linalg.norm(got - exp) / np.linalg.norm(exp)
    print("Relative error:", rel)



# revision 5
# speedup vs baseline: 1.2858x; 1.2858x over previous
"""Trainium2 Bass kernel for nn_CrossAttention_82471962018390.

Dilated (d=2) 9x9 neighborhood cross-attention, q 48x48 vs k/v 24x24.

Math identity used: the nearest-exact 2x upsample + dilation-2 NATTEN window
collapses so that query (h, w) attends to the ORIGINAL 24x24 k/v grid at
rows clip(h//2-4, 0, 15) + 0..8, cols clip(w//2-4, 0, 15) + 0..8 (a
contiguous 9x9 window; the 4 queries in each 2x2 block share one window).

Kernel structure (per (b, head) pair; 2 pairs per core, 8 cores = 16 pairs):
  - 16 row-bands by s_h = clip(h//2-4,0,15); band s attends the 9x24=216-key
    slab k[:, s:s+9, :].
  - Scores computed transposed: S^T[key, query] = (K slab)^T @ Q, with the
    column-window mask folded INTO the matmul via 16 extra contraction rows:
    lhsT rows 64:80 hold M0[r, kw] (0 or -30), rhs rows 64:80 hold the
    one-hot of s_w(w).  scale 1/8 is folded into q on the host.
  - exp on ScalarE (PSUM -> SBUF bf16).
  - One PV matmul per key-chunk with stationary [V^T | ones*64] giving
    [PV (64 rows); sumexp replicated (64 rows)] in one PSUM tile.
  - One VectorE divide -> output tile, DMA out.
Bands are grouped (10/2*5/2*5/2*4/10 h-rows) so ACT/DVE instructions run on
~480-column tiles.
"""

import numpy as np
import ml_dtypes

try:
    import concourse.bass as bass
    import concourse.bacc as bacc
    import concourse.tile as tile
    from concourse import mybir
    from concourse.bass_utils import run_bass_kernel_spmd
except ImportError:  # pragma: no cover
    import sys

    sys.path.insert(0, "/opt/trn_rl_repo")
    import concourse.bass as bass
    import concourse.bacc as bacc
    import concourse.tile as tile
    from concourse import mybir
    from concourse.bass_utils import run_bass_kernel_spmd

from contextlib import ExitStack

BF = ml_dtypes.bfloat16
N_CORES = 8
NPAIR = 2  # (b, head) pairs per core
DH = 64
HQ = WQ = 48
HK = WK = 24
NQ = HQ * WQ  # 2304
NK = HK * WK  # 576
BAND_KEYS = 9 * WK  # 216
CH1 = 128  # keys in chunk 1 of a band
CH2 = BAND_KEYS - CH1  # 88

# s(i) = clip(i//2 - 4, 0, 15) for i in 0..47
_S = np.clip(np.arange(48) // 2 - 4, 0, 15)

# Band groups: (q column offset, width, [(band s, rel q offset, band width)])
# band s covers h rows where s_h(h) == s: s=0 -> h 0..9, s=1..14 -> 2 rows,
# s=15 -> h 38..47.
def _groups():
    bands_h0 = {}
    for h in range(48):
        bands_h0.setdefault(int(_S[h]), []).append(h)
    spans = {s: (hs[0], len(hs)) for s, hs in bands_h0.items()}
    layout = [[0], [1, 2, 3, 4, 5], [6, 7, 8, 9, 10], [11, 12, 13, 14], [15]]
    groups = []
    for g in layout:
        h0 = spans[g[0]][0]
        width = sum(spans[s][1] for s in g) * 48
        bands = []
        off = 0
        for s in g:
            bw = spans[s][1] * 48
            bands.append((s, off, bw))
            off += bw
        groups.append((h0 * 48, width, bands))
    return groups


GROUPS = _groups()
MAXM = max(w for _, w, _ in GROUPS)  # 480


def _host_tables():
    """M0exp [16, 576] and Bw [16, 2304] mask/one-hot tables (fp32)."""
    m0 = np.full((16, WK), -30.0, np.float32)
    for r in range(16):
        m0[r, r : r + 9] = 0.0
    m0exp = np.tile(m0[:, None, :], (1, HK, 1)).reshape(16, NK)
    bw = np.zeros((16, NQ), np.float32)
    for w in range(48):
        bw[_S[w], np.arange(48) * 48 + w] = 1.0
    return m0exp, bw


def build_kernel(ctx: ExitStack, tc, qb, km, vb, out):
    nc = tc.nc
    FP32 = mybir.dt.float32
    BF16 = mybir.dt.bfloat16
    Exp = mybir.ActivationFunctionType.Exp

    qpool = ctx.enter_context(tc.tile_pool(name="qb", bufs=2))
    kpool = ctx.enter_context(tc.tile_pool(name="km", bufs=2))
    vpool = ctx.enter_context(tc.tile_pool(name="vt", bufs=1))
    spool = ctx.enter_context(tc.tile_pool(name="scores", bufs=2, space="PSUM"))
    opool = ctx.enter_context(tc.tile_pool(name="opsum", bufs=2, space="PSUM"))
    epool = ctx.enter_context(tc.tile_pool(name="expo", bufs=2))
    rpool = ctx.enter_context(tc.tile_pool(name="res", bufs=2))

    # Persistent V'' stationary tiles, one per pair: 16 bands x 2 chunks of
    # [128, 128] ([V^T chunk | ones]) packed side by side, filled by two big
    # contiguous DMAs each (ones baked into the DRAM image by the host).
    # Issued on the otherwise-idle GpSimd queue.
    vts = []
    for p in range(NPAIR):
        vt = vpool.tile([128, 4096], BF16, tag=f"vt{p}", name=f"vt{p}")
        nc.gpsimd.dma_start(vt[:, 0:2048], vb[:, 4096 * p : 4096 * p + 2048])
        nc.gpsimd.dma_start(vt[:, 2048:4096], vb[:, 4096 * p + 2048 : 4096 * p + 4096])
        vts.append(vt)

    for p in range(NPAIR):
        qb_t = qpool.tile([80, NQ], BF16)
        nc.sync.dma_start(qb_t[:, :], qb[80 * p : 80 * p + 80, :])
        km_t = kpool.tile([80, NK], BF16)
        nc.sync.dma_start(km_t[:, :], km[80 * p : 80 * p + 80, :])
        vt = vts[p]

        for (q0, M, bands) in GROUPS:
            s1 = spool.tile([128, MAXM], FP32, tag="s1")
            s2 = spool.tile([128, MAXM], FP32, tag="s2")
            for (s, off, bw) in bands:
                rhs = qb_t[:, q0 + off : q0 + off + bw]
                nc.tensor.matmul(
                    s1[:, off : off + bw],
                    km_t[:, 24 * s : 24 * s + CH1],
                    rhs,
                    start=True,
                    stop=True,
                )
                nc.tensor.matmul(
                    s2[0:CH2, off : off + bw],
                    km_t[:, 24 * s + CH1 : 24 * s + BAND_KEYS],
                    rhs,
                    start=True,
                    stop=True,
                )
            e1 = epool.tile([128, MAXM], BF16, tag="e1")
            e2 = epool.tile([128, MAXM], BF16, tag="e2")
            nc.scalar.activation(e1[:, :M], s1[:, :M], Exp)
            nc.scalar.activation(e2[0:CH2, :M], s2[0:CH2, :M], Exp)

            o = opool.tile([128, MAXM], FP32, tag="o")
            for (s, off, bw) in bands:
                # Same-output WAW dep keeps the accumulation pair ordered;
                # CoreSim's psum-group check validates the final schedule.
                nc.tensor.matmul(
                    o[:, off : off + bw],
                    vt[:, 256 * s : 256 * s + 128],
                    e1[:, off : off + bw],
                    start=True,
                    stop=False,
                )
                nc.tensor.matmul(
                    o[:, off : off + bw],
                    vt[0:CH2, 256 * s + 128 : 256 * s + 256],
                    e2[0:CH2, off : off + bw],
                    start=False,
                    stop=True,
                )
            # DVE can read only ONE PSUM operand per instruction: reciprocal
            # of the replicated sumexp rows PSUM->SBUF, then PV * recip.
            # ~18 bits of accuracy is far inside the 2e-2 tolerance.
            rcp = rpool.tile([64, MAXM], FP32, tag="rcp")
            nc.vector.reciprocal(rcp[:, :M], o[64:128, :M])
            res = rpool.tile([64, MAXM], FP32, tag="res")
            nc.vector.tensor_mul(res[:, :M], o[0:64, :M], rcp[:, :M])
            nc.sync.dma_start(out[64 * p : 64 * p + 64, q0 : q0 + M], res[:, :M])


_CACHE = {}


def _get_nc():
    if "nc" not in _CACHE:
        nc = bacc.Bacc(
            "TRN2", target_bir_lowering=False, debug=False, num_devices=N_CORES
        )
        qb = nc.dram_tensor(
            "qb", [NPAIR * 80, NQ], mybir.dt.bfloat16, kind="ExternalInput"
        ).ap()
        km = nc.dram_tensor(
            "km", [NPAIR * 80, NK], mybir.dt.bfloat16, kind="ExternalInput"
        ).ap()
        vb = nc.dram_tensor(
            "vb", [128, NPAIR * 4096], mybir.dt.bfloat16, kind="ExternalInput"
        ).ap()
        out = nc.dram_tensor(
            "out", [NPAIR * 64, NQ], mybir.dt.float32, kind="ExternalOutput"
        ).ap()
        with tile.TileContext(nc) as tc, ExitStack() as ctx:
            build_kernel(ctx, tc, qb, km, vb, out)
        nc.compile()
        _CACHE["nc"] = nc
    return _CACHE["nc"]


def kernel(q: np.ndarray, k: np.ndarray, v: np.ndarray) -> np.ndarray:
    assert q.shape == (2, 512, HQ, WQ) and k.shape == (2, 512, HK, WK)
    m0exp, bw = _host_tables()
    nc = _get_nc()

    in_maps = []
    for c in range(N_CORES):
        qbc = np.empty((NPAIR * 80, NQ), BF)
        kmc = np.empty((NPAIR * 80, NK), BF)
        vbc = np.zeros((128, NPAIR * 4096), BF)
        for pl in range(NPAIR):
            pg = NPAIR * c + pl
            b, hd = pg // 8, pg % 8
            qbc[80 * pl : 80 * pl + 64] = (
                q[b, 64 * hd : 64 * hd + 64].reshape(64, NQ) / 8.0
            ).astype(BF)
            qbc[80 * pl + 64 : 80 * pl + 80] = bw.astype(BF)
            kmc[80 * pl : 80 * pl + 64] = (
                k[b, 64 * hd : 64 * hd + 64].reshape(64, NK).astype(BF)
            )
            kmc[80 * pl + 64 : 80 * pl + 80] = m0exp.astype(BF)
            v3 = v[b, 64 * hd : 64 * hd + 64].reshape(64, HK, WK)
            for s in range(16):
                slab = v3[:, s : s + 9, :].reshape(64, BAND_KEYS).T.astype(BF)
                base = 4096 * pl + 256 * s
                vbc[:, base : base + 64] = slab[0:CH1]
                vbc[:, base + 64 : base + 128] = 1.0
                vbc[0:CH2, base + 128 : base + 192] = slab[CH1:BAND_KEYS]
                vbc[0:CH2, base + 192 : base + 256] = 1.0
        in_maps.append({"qb": qbc, "km": kmc, "vb": vbc})

    results = run_bass_kernel_spmd(nc, in_maps, list(range(N_CORES))).results

    out = np.empty((2, 512, HQ, WQ), np.float32)
    for c in range(N_CORES):
        o = results[c]["out"]
        for pl in range(NPAIR):
            pg = NPAIR * c + pl
            b, hd = pg // 8, pg % 8
            out[b, 64 * hd : 64 * hd + 64] = o[64 * pl : 64 * pl + 64].reshape(
                64, HQ, WQ
            )
    return out


if __name__ == "__main__":
    qq = np.load("/root/problem/q.npy")
    kk = np.load("/root/problem/k.npy")
    vv = np.load("/root/problem/v.npy")
    got = kernel(qq, kk, vv)
    exp = np.load("/root/problem/expected.npy")
    rel = np.linalg.norm(got - exp) / np.linalg.norm(exp)
    print("Relative error:", rel)



# revision 10
# speedup vs baseline: 1.4829x; 1.1533x over previous
"""Trainium2 Bass kernel for nn_CrossAttention_82471962018390.

Dilated (d=2) 9x9 neighborhood cross-attention, q 48x48 vs k/v 24x24.

Math identity used: the nearest-exact 2x upsample + dilation-2 NATTEN window
collapses so that query (h, w) attends to the ORIGINAL 24x24 k/v grid at
rows clip(h//2-4, 0, 15) + 0..8, cols clip(w//2-4, 0, 15) + 0..8 (a
contiguous 9x9 window; the 4 queries in each 2x2 block share one window).

Kernel structure (per (b, head) pair; 2 pairs per core, 8 cores = 16 pairs):
  - 16 row-bands by s_h = clip(h//2-4,0,15); band s attends the 9x24=216-key
    slab k[:, s:s+9, :].
  - Scores computed transposed: S^T[key, query] = (K slab)^T @ Q, with the
    column-window mask folded INTO the matmul via 16 extra contraction rows:
    lhsT rows 64:80 hold M0[r, kw] (0 or -30), rhs rows 64:80 hold the
    one-hot of s_w(w).  scale 1/8 is folded into q on the host.
  - exp on ScalarE (PSUM -> SBUF bf16).
  - One PV matmul per key-chunk with stationary [V^T | ones*64] giving
    [PV (64 rows); sumexp replicated (64 rows)] in one PSUM tile.
  - One VectorE divide -> output tile, DMA out.
Bands are grouped (10/2*5/2*5/2*4/10 h-rows) so ACT/DVE instructions run on
~480-column tiles.
"""

import numpy as np
import ml_dtypes

try:
    import concourse.bass as bass
    import concourse.bacc as bacc
    import concourse.tile as tile
    from concourse import mybir
    from concourse.bass_utils import run_bass_kernel_spmd
except ImportError:  # pragma: no cover
    import sys

    sys.path.insert(0, "/opt/trn_rl_repo")
    import concourse.bass as bass
    import concourse.bacc as bacc
    import concourse.tile as tile
    from concourse import mybir
    from concourse.bass_utils import run_bass_kernel_spmd

from contextlib import ExitStack

BF = ml_dtypes.bfloat16
N_CORES = 8
NPAIR = 2  # (b, head) pairs per core
DH = 64
HQ = WQ = 48
HK = WK = 24
NQ = HQ * WQ  # 2304
NK = HK * WK  # 576
BAND_KEYS = 9 * WK  # 216
CH1 = 128  # keys in chunk 1 of a band
CH2 = BAND_KEYS - CH1  # 88

# s(i) = clip(i//2 - 4, 0, 15) for i in 0..47
_S = np.clip(np.arange(48) // 2 - 4, 0, 15)

# Band groups: (q column offset, width, [(band s, rel q offset, band width)])
# band s covers h rows where s_h(h) == s: s=0 -> h 0..9, s=1..14 -> 2 rows,
# s=15 -> h 38..47.
def _groups():
    bands_h0 = {}
    for h in range(48):
        bands_h0.setdefault(int(_S[h]), []).append(h)
    spans = {s: (hs[0], len(hs)) for s, hs in bands_h0.items()}
    layout = [[0], [1, 2, 3, 4, 5], [6, 7, 8, 9, 10], [11, 12, 13, 14], [15]]
    groups = []
    for g in layout:
        h0 = spans[g[0]][0]
        width = sum(spans[s][1] for s in g) * 48
        bands = []
        off = 0
        for s in g:
            bw = spans[s][1] * 48
            bands.append((s, off, bw))
            off += bw
        groups.append((h0 * 48, width, bands))
    return groups


GROUPS = _groups()
MAXM = max(w for _, w, _ in GROUPS)  # 480


def _host_tables():
    """M0exp [16, 576] and Bw [16, 2304] mask/one-hot tables (fp32)."""
    m0 = np.full((16, WK), -30.0, np.float32)
    for r in range(16):
        m0[r, r : r + 9] = 0.0
    m0exp = np.tile(m0[:, None, :], (1, HK, 1)).reshape(16, NK)
    bw = np.zeros((16, NQ), np.float32)
    for w in range(48):
        bw[_S[w], np.arange(48) * 48 + w] = 1.0
    return m0exp, bw


def build_kernel(ctx: ExitStack, tc, qb, km, vb, out):
    nc = tc.nc
    FP32 = mybir.dt.float32
    BF16 = mybir.dt.bfloat16
    Exp = mybir.ActivationFunctionType.Exp
    Ln = mybir.ActivationFunctionType.Ln

    qpool = ctx.enter_context(tc.tile_pool(name="qb", bufs=2))
    kpool = ctx.enter_context(tc.tile_pool(name="km", bufs=2))
    vpool = ctx.enter_context(tc.tile_pool(name="vt", bufs=1))
    spool = ctx.enter_context(tc.tile_pool(name="scores", bufs=2, space="PSUM"))
    opool = ctx.enter_context(tc.tile_pool(name="opsum", bufs=2, space="PSUM"))
    epool = ctx.enter_context(tc.tile_pool(name="expo", bufs=2))
    rpool = ctx.enter_context(tc.tile_pool(name="res", bufs=2))

    # Persistent V'' stationary tiles, one per pair: 16 bands x 2 chunks of
    # [128, 128] ([V^T chunk | ones]) packed side by side, filled by two big
    # contiguous DMAs each (ones baked into the DRAM image by the host).
    # Issued on the otherwise-idle GpSimd queue.
    vts = []
    for p in range(NPAIR):
        vt = vpool.tile([128, 4096], BF16, tag=f"vt{p}", name=f"vt{p}")
        nc.gpsimd.dma_start(vt[:, 0:2048], vb[:, 4096 * p : 4096 * p + 2048])
        nc.gpsimd.dma_start(vt[:, 2048:4096], vb[:, 4096 * p + 2048 : 4096 * p + 4096])
        vts.append(vt)

    for p in range(NPAIR):
        qb_t = qpool.tile([80, NQ], BF16)
        nc.sync.dma_start(qb_t[:, :], qb[80 * p : 80 * p + 80, :])
        km_t = kpool.tile([80, NK], BF16)
        nc.sync.dma_start(km_t[:, :], km[80 * p : 80 * p + 80, :])
        vt = vts[p]

        for gi, (q0, M, bands) in enumerate(GROUPS):
            s1 = spool.tile([128, MAXM], FP32, tag="s1")
            s2 = spool.tile([128, MAXM], FP32, tag="s2")
            for (s, off, bw) in bands:
                rhs = qb_t[:, q0 + off : q0 + off + bw]
                nc.tensor.matmul(
                    s1[:, off : off + bw],
                    km_t[:, 24 * s : 24 * s + CH1],
                    rhs,
                    start=True,
                    stop=True,
                )
                nc.tensor.matmul(
                    s2[0:CH2, off : off + bw],
                    km_t[:, 24 * s + CH1 : 24 * s + BAND_KEYS],
                    rhs,
                    start=True,
                    stop=True,
                )
            e1 = epool.tile([128, MAXM], BF16, tag="e1")
            e2 = epool.tile([128, MAXM], BF16, tag="e2")
            nc.scalar.activation(e1[:, :M], s1[:, :M], Exp)
            nc.scalar.activation(e2[0:CH2, :M], s2[0:CH2, :M], Exp)

            o = opool.tile([128, MAXM], FP32, tag="o")
            for (s, off, bw) in bands:
                # Same-output WAW dep keeps the accumulation pair ordered;
                # CoreSim's psum-group check validates the final schedule.
                nc.tensor.matmul(
                    o[:, off : off + bw],
                    vt[:, 256 * s : 256 * s + 128],
                    e1[:, off : off + bw],
                    start=True,
                    stop=False,
                )
                nc.tensor.matmul(
                    o[:, off : off + bw],
                    vt[0:CH2, 256 * s + 128 : 256 * s + 256],
                    e2[0:CH2, off : off + bw],
                    start=False,
                    stop=True,
                )
            # DVE can read only ONE PSUM operand per instruction: reciprocal
            # of the replicated sumexp rows PSUM->SBUF, then PV * recip.
            # DVE's iterative reciprocal is ~6.6ns/col while ScalarE can do
            # 1/x = exp(-ln x) at ~1.7ns/col (ln+exp share one table set);
            # split groups between the engines to balance their queues.
            rcp = rpool.tile([64, MAXM], FP32, tag="rcp")
            if (gi + p) % 2 == 0:
                lnm = rpool.tile([64, MAXM], FP32, tag="lnm")
                nc.scalar.activation(lnm[:, :M], o[64:128, :M], Ln)
                nc.scalar.activation(rcp[:, :M], lnm[:, :M], Exp, scale=-1.0)
            else:
                nc.vector.reciprocal(rcp[:, :M], o[64:128, :M])
            res = rpool.tile([64, MAXM], FP32, tag="res")
            nc.vector.tensor_mul(res[:, :M], o[0:64, :M], rcp[:, :M])
            nc.sync.dma_start(out[64 * p : 64 * p + 64, q0 : q0 + M], res[:, :M])


_CACHE = {}


def _get_nc():
    if "nc" not in _CACHE:
        nc = bacc.Bacc(
            "TRN2", target_bir_lowering=False, debug=False, num_devices=N_CORES
        )
        qb = nc.dram_tensor(
            "qb", [NPAIR * 80, NQ], mybir.dt.bfloat16, kind="ExternalInput"
        ).ap()
        km = nc.dram_tensor(
            "km", [NPAIR * 80, NK], mybir.dt.bfloat16, kind="ExternalInput"
        ).ap()
        vb = nc.dram_tensor(
            "vb", [128, NPAIR * 4096], mybir.dt.bfloat16, kind="ExternalInput"
        ).ap()
        out = nc.dram_tensor(
            "out", [NPAIR * 64, NQ], mybir.dt.float32, kind="ExternalOutput"
        ).ap()
        with tile.TileContext(nc) as tc, ExitStack() as ctx:
            build_kernel(ctx, tc, qb, km, vb, out)
        nc.compile()
        _CACHE["nc"] = nc
    return _CACHE["nc"]


def kernel(q: np.ndarray, k: np.ndarray, v: np.ndarray) -> np.ndarray:
    assert q.shape == (2, 512, HQ, WQ) and k.shape == (2, 512, HK, WK)
    m0exp, bw = _host_tables()
    nc = _get_nc()

    in_maps = []
    for c in range(N_CORES):
        qbc = np.empty((NPAIR * 80, NQ), BF)
        kmc = np.empty((NPAIR * 80, NK), BF)
        vbc = np.zeros((128, NPAIR * 4096), BF)
        for pl in range(NPAIR):
            pg = NPAIR * c + pl
            b, hd = pg // 8, pg % 8
            qbc[80 * pl : 80 * pl + 64] = (
                q[b, 64 * hd : 64 * hd + 64].reshape(64, NQ) / 8.0
            ).astype(BF)
            qbc[80 * pl + 64 : 80 * pl + 80] = bw.astype(BF)
            kmc[80 * pl : 80 * pl + 64] = (
                k[b, 64 * hd : 64 * hd + 64].reshape(64, NK).astype(BF)
            )
            kmc[80 * pl + 64 : 80 * pl + 80] = m0exp.astype(BF)
            v3 = v[b, 64 * hd : 64 * hd + 64].reshape(64, HK, WK)
            for s in range(16):
                slab = v3[:, s : s + 9, :].reshape(64, BAND_KEYS).T.astype(BF)
                base = 4096 * pl + 256 * s
                vbc[:, base : base + 64] = slab[0:CH1]
                vbc[:, base + 64 : base + 128] = 1.0
                vbc[0:CH2, base + 128 : base + 192] = slab[CH1:BAND_KEYS]
                vbc[0:CH2, base + 192 : base + 256] = 1.0
        in_maps.append({"qb": qbc, "km": kmc, "vb": vbc})

    results = run_bass_kernel_spmd(nc, in_maps, list(range(N_CORES))).results

    out = np.empty((2, 512, HQ, WQ), np.float32)
    for c in range(N_CORES):
        o = results[c]["out"]
        for pl in range(NPAIR):
            pg = NPAIR * c + pl
            b, hd = pg // 8, pg % 8
            out[b, 64 * hd : 64 * hd + 64] = o[64 * pl : 64 * pl + 64].reshape(
                64, HQ, WQ
            )
    return out


if __name__ == "__main__":
    qq = np.load("/root/problem/q.npy")
    kk = np.load("/root/problem/k.npy")
    vv = np.load("/root/problem/v.npy")
    got = kernel(qq, kk, vv)
    exp = np.load("/root/problem/expected.npy")
    rel = np.linalg.norm(got - exp) / np.linalg.norm(exp)
    print("Relative error:", rel)



# revision 11
# speedup vs baseline: 1.6130x; 1.0877x over previous
"""Trainium2 Bass kernel for nn_CrossAttention_82471962018390.

Dilated (d=2) 9x9 neighborhood cross-attention, q 48x48 vs k/v 24x24.

Math identity used: the nearest-exact 2x upsample + dilation-2 NATTEN window
collapses so that query (h, w) attends to the ORIGINAL 24x24 k/v grid at
rows clip(h//2-4, 0, 15) + 0..8, cols clip(w//2-4, 0, 15) + 0..8 (a
contiguous 9x9 window; the 4 queries in each 2x2 block share one window).

Kernel structure (per (b, head) pair; 2 pairs per core, 8 cores = 16 pairs):
  - 16 row-bands by s_h = clip(h//2-4,0,15); band s attends the 9x24=216-key
    slab k[:, s:s+9, :].
  - Scores computed transposed: S^T[key, query] = (K slab)^T @ Q, with the
    column-window mask folded INTO the matmul via 16 extra contraction rows:
    lhsT rows 64:80 hold M0[r, kw] (0 or -30), rhs rows 64:80 hold the
    one-hot of s_w(w).  scale 1/8 is folded into q on the host.
  - exp on ScalarE (PSUM -> SBUF bf16).
  - One PV matmul per key-chunk with stationary [V^T | ones*64] giving
    [PV (64 rows); sumexp replicated (64 rows)] in one PSUM tile.
  - One VectorE divide -> output tile, DMA out.
Bands are grouped (10/2*5/2*5/2*4/10 h-rows) so ACT/DVE instructions run on
~480-column tiles.
"""

import numpy as np
import ml_dtypes

try:
    import concourse.bass as bass
    import concourse.bacc as bacc
    import concourse.tile as tile
    from concourse import mybir
    from concourse.bass_utils import run_bass_kernel_spmd
except ImportError:  # pragma: no cover
    import sys

    sys.path.insert(0, "/opt/trn_rl_repo")
    import concourse.bass as bass
    import concourse.bacc as bacc
    import concourse.tile as tile
    from concourse import mybir
    from concourse.bass_utils import run_bass_kernel_spmd

from contextlib import ExitStack

BF = ml_dtypes.bfloat16
N_CORES = 8
NPAIR = 2  # (b, head) pairs per core
DH = 64
HQ = WQ = 48
HK = WK = 24
NQ = HQ * WQ  # 2304
NK = HK * WK  # 576
BAND_KEYS = 9 * WK  # 216
CH1 = 128  # keys in chunk 1 of a band
CH2 = BAND_KEYS - CH1  # 88

# s(i) = clip(i//2 - 4, 0, 15) for i in 0..47
_S = np.clip(np.arange(48) // 2 - 4, 0, 15)

# Band groups: (q column offset, width, [(band s, rel q offset, band width)])
# band s covers h rows where s_h(h) == s: s=0 -> h 0..9, s=1..14 -> 2 rows,
# s=15 -> h 38..47.
def _groups():
    bands_h0 = {}
    for h in range(48):
        bands_h0.setdefault(int(_S[h]), []).append(h)
    spans = {s: (hs[0], len(hs)) for s, hs in bands_h0.items()}
    layout = [[0], [1, 2, 3, 4, 5], [6, 7, 8, 9, 10], [11, 12, 13, 14], [15]]
    groups = []
    for g in layout:
        h0 = spans[g[0]][0]
        width = sum(spans[s][1] for s in g) * 48
        bands = []
        off = 0
        for s in g:
            bw = spans[s][1] * 48
            bands.append((s, off, bw))
            off += bw
        groups.append((h0 * 48, width, bands))
    return groups


GROUPS = _groups()
MAXM = max(w for _, w, _ in GROUPS)  # 480


def _host_tables():
    """M0exp [16, 576] and Bw [16, 2304] mask/one-hot tables (fp32)."""
    m0 = np.full((16, WK), -30.0, np.float32)
    for r in range(16):
        m0[r, r : r + 9] = 0.0
    m0exp = np.tile(m0[:, None, :], (1, HK, 1)).reshape(16, NK)
    bw = np.zeros((16, NQ), np.float32)
    for w in range(48):
        bw[_S[w], np.arange(48) * 48 + w] = 1.0
    return m0exp, bw


def build_kernel(ctx: ExitStack, tc, qb, km, vb, out):
    nc = tc.nc
    FP32 = mybir.dt.float32
    BF16 = mybir.dt.bfloat16
    Exp = mybir.ActivationFunctionType.Exp
    Ln = mybir.ActivationFunctionType.Ln

    # Preload the one ACT table set holding BOTH exp and ln; without this the
    # compiler's per-activation set picker thrashes between exp_and_others and
    # natural_log (~1.3us per reload, 2 reloads per ln-based reciprocal).
    from concourse.hw_specs import get_activation_tables

    set_id = list(get_activation_tables(nc.m.arch)).index("natural_log_exp_and_others")
    nc.scalar.add_instruction(
        mybir.InstLoadActFuncSet(
            name=nc.get_next_instruction_name(), act_func_set_id=set_id
        )
    )

    qpool = ctx.enter_context(tc.tile_pool(name="qb", bufs=2))
    kpool = ctx.enter_context(tc.tile_pool(name="km", bufs=2))
    vpool = ctx.enter_context(tc.tile_pool(name="vt", bufs=1))
    spool = ctx.enter_context(tc.tile_pool(name="scores", bufs=2, space="PSUM"))
    opool = ctx.enter_context(tc.tile_pool(name="opsum", bufs=2, space="PSUM"))
    epool = ctx.enter_context(tc.tile_pool(name="expo", bufs=2))
    rpool = ctx.enter_context(tc.tile_pool(name="res", bufs=2))

    # Persistent V'' stationary tiles, one per pair: 16 bands x 2 chunks of
    # [128, 128] ([V^T chunk | ones]) packed side by side, filled by two big
    # contiguous DMAs each (ones baked into the DRAM image by the host).
    # Issued on the otherwise-idle GpSimd queue.
    vts = []
    for p in range(NPAIR):
        vt = vpool.tile([128, 4096], BF16, tag=f"vt{p}", name=f"vt{p}")
        nc.gpsimd.dma_start(vt[:, 0:2048], vb[:, 4096 * p : 4096 * p + 2048])
        nc.gpsimd.dma_start(vt[:, 2048:4096], vb[:, 4096 * p + 2048 : 4096 * p + 4096])
        vts.append(vt)

    for p in range(NPAIR):
        qb_t = qpool.tile([80, NQ], BF16)
        nc.sync.dma_start(qb_t[:, :], qb[80 * p : 80 * p + 80, :])
        km_t = kpool.tile([80, NK], BF16)
        nc.sync.dma_start(km_t[:, :], km[80 * p : 80 * p + 80, :])
        vt = vts[p]

        for gi, (q0, M, bands) in enumerate(GROUPS):
            s1 = spool.tile([128, MAXM], FP32, tag="s1")
            s2 = spool.tile([128, MAXM], FP32, tag="s2")
            for (s, off, bw) in bands:
                rhs = qb_t[:, q0 + off : q0 + off + bw]
                nc.tensor.matmul(
                    s1[:, off : off + bw],
                    km_t[:, 24 * s : 24 * s + CH1],
                    rhs,
                    start=True,
                    stop=True,
                )
                nc.tensor.matmul(
                    s2[0:CH2, off : off + bw],
                    km_t[:, 24 * s + CH1 : 24 * s + BAND_KEYS],
                    rhs,
                    start=True,
                    stop=True,
                )
            e1 = epool.tile([128, MAXM], BF16, tag="e1")
            e2 = epool.tile([128, MAXM], BF16, tag="e2")
            nc.scalar.activation(e1[:, :M], s1[:, :M], Exp)
            nc.scalar.activation(e2[0:CH2, :M], s2[0:CH2, :M], Exp)

            o = opool.tile([128, MAXM], FP32, tag="o")
            for (s, off, bw) in bands:
                # Same-output WAW dep keeps the accumulation pair ordered;
                # CoreSim's psum-group check validates the final schedule.
                nc.tensor.matmul(
                    o[:, off : off + bw],
                    vt[:, 256 * s : 256 * s + 128],
                    e1[:, off : off + bw],
                    start=True,
                    stop=False,
                )
                nc.tensor.matmul(
                    o[:, off : off + bw],
                    vt[0:CH2, 256 * s + 128 : 256 * s + 256],
                    e2[0:CH2, off : off + bw],
                    start=False,
                    stop=True,
                )
            # DVE can read only ONE PSUM operand per instruction: reciprocal
            # of the replicated sumexp rows PSUM->SBUF, then PV * recip.
            # DVE's iterative reciprocal is ~6.6ns/col while ScalarE can do
            # 1/x = exp(-ln x) at ~1.7ns/col (ln+exp share one table set);
            # split groups between the engines to balance their queues.
            rcp = rpool.tile([64, MAXM], FP32, tag="rcp")
            if (gi + p) % 2 == 0:
                lnm = rpool.tile([64, MAXM], FP32, tag="lnm")
                nc.scalar.activation(lnm[:, :M], o[64:128, :M], Ln)
                nc.scalar.activation(rcp[:, :M], lnm[:, :M], Exp, scale=-1.0)
            else:
                nc.vector.reciprocal(rcp[:, :M], o[64:128, :M])
            res = rpool.tile([64, MAXM], FP32, tag="res")
            nc.vector.tensor_mul(res[:, :M], o[0:64, :M], rcp[:, :M])
            nc.sync.dma_start(out[64 * p : 64 * p + 64, q0 : q0 + M], res[:, :M])


_CACHE = {}


def _get_nc():
    if "nc" not in _CACHE:
        nc = bacc.Bacc(
            "TRN2", target_bir_lowering=False, debug=False, num_devices=N_CORES
        )
        qb = nc.dram_tensor(
            "qb", [NPAIR * 80, NQ], mybir.dt.bfloat16, kind="ExternalInput"
        ).ap()
        km = nc.dram_tensor(
            "km", [NPAIR * 80, NK], mybir.dt.bfloat16, kind="ExternalInput"
        ).ap()
        vb = nc.dram_tensor(
            "vb", [128, NPAIR * 4096], mybir.dt.bfloat16, kind="ExternalInput"
        ).ap()
        out = nc.dram_tensor(
            "out", [NPAIR * 64, NQ], mybir.dt.float32, kind="ExternalOutput"
        ).ap()
        with tile.TileContext(nc) as tc, ExitStack() as ctx:
            build_kernel(ctx, tc, qb, km, vb, out)
        nc.compile()
        _CACHE["nc"] = nc
    return _CACHE["nc"]


def kernel(q: np.ndarray, k: np.ndarray, v: np.ndarray) -> np.ndarray:
    assert q.shape == (2, 512, HQ, WQ) and k.shape == (2, 512, HK, WK)
    m0exp, bw = _host_tables()
    nc = _get_nc()

    in_maps = []
    for c in range(N_CORES):
        qbc = np.empty((NPAIR * 80, NQ), BF)
        kmc = np.empty((NPAIR * 80, NK), BF)
        vbc = np.zeros((128, NPAIR * 4096), BF)
        for pl in range(NPAIR):
            pg = NPAIR * c + pl
            b, hd = pg // 8, pg % 8
            qbc[80 * pl : 80 * pl + 64] = (
                q[b, 64 * hd : 64 * hd + 64].reshape(64, NQ) / 8.0
            ).astype(BF)
            qbc[80 * pl + 64 : 80 * pl + 80] = bw.astype(BF)
            kmc[80 * pl : 80 * pl + 64] = (
                k[b, 64 * hd : 64 * hd + 64].reshape(64, NK).astype(BF)
            )
            kmc[80 * pl + 64 : 80 * pl + 80] = m0exp.astype(BF)
            v3 = v[b, 64 * hd : 64 * hd + 64].reshape(64, HK, WK)
            for s in range(16):
                slab = v3[:, s : s + 9, :].reshape(64, BAND_KEYS).T.astype(BF)
                base = 4096 * pl + 256 * s
                vbc[:, base : base + 64] = slab[0:CH1]
                vbc[:, base + 64 : base + 128] = 1.0
                vbc[0:CH2, base + 128 : base + 192] = slab[CH1:BAND_KEYS]
                vbc[0:CH2, base + 192 : base + 256] = 1.0
        in_maps.append({"qb": qbc, "km": kmc, "vb": vbc})

    results = run_bass_kernel_spmd(nc, in_maps, list(range(N_CORES))).results

    out = np.empty((2, 512, HQ, WQ), np.float32)
    for c in range(N_CORES):
        o = results[c]["out"]
        for pl in range(NPAIR):
            pg = NPAIR * c + pl
            b, hd = pg // 8, pg % 8
            out[b, 64 * hd : 64 * hd + 64] = o[64 * pl : 64 * pl + 64].reshape(
                64, HQ, WQ
            )
    return out


if __name__ == "__main__":
    qq = np.load("/root/problem/q.npy")
    kk = np.load("/root/problem/k.npy")
    vv = np.load("/root/problem/v.npy")
    got = kernel(qq, kk, vv)
    exp = np.load("/root/problem/expected.npy")
    rel = np.linalg.norm(got - exp) / np.linalg.norm(exp)
    print("Relative error:", rel)



# revision 12
# speedup vs baseline: 1.7469x; 1.0830x over previous
"""Trainium2 Bass kernel for nn_CrossAttention_82471962018390.

Dilated (d=2) 9x9 neighborhood cross-attention, q 48x48 vs k/v 24x24.

Math identity: the nearest-exact 2x upsample + dilation-2 NATTEN window
collapses so query (h, w) attends the ORIGINAL 24x24 k/v grid at rows
clip(h//2-4,0,15)+0..8, cols clip(w//2-4,0,15)+0..8.

Structure (per core: 2 (b, head) pairs; 8 cores = 16 pairs):
  - 8 MERGED BANDS m = band pair (2m, 2m+1): key rows 2m..2m+9 (10 rows).
    Row-window membership folded into the matmul with 16 extra contraction
    rows (one-hot of s_h x M1 row-mask), on top of the 16 col-mask rows.
    Contraction = 64 dh + 16 colmask + 16 rowmask = 96.
  - Each merged band splits keys by column into overlapping chunks
    A = cols 0..11, B = cols 12..23 (120 keys each, padded to 128 with
    fully-masked keys).  Queries w0..15 need only A, w32..47 only B,
    w16..31 both -> 1.33 avg chunks/query instead of 2.
  - Query columns are laid out w-block-major: col = wb*768 + h*16 + wi,
    so each (chunk, wb-range) streams as one strided matmul rhs.
  - exp on ScalarE PSUM->SBUF bf16.
  - PV and sumexp(via ones stationary) matmuls are PAIR-STACKED: pair0 in
    PSUM partitions 0:64, pair1 in 64:128, so the softmax normalization
    (reciprocal + multiply) runs on all 128 DVE lanes, halving its column
    count.  Chunk A writes (wb0|wb1), chunk B (wb1|wb2) accumulating via
    PSUM per-element has_written bits.
  - 1/sumexp split between ScalarE (exp(-ln x), both fns in the
    natural_log_exp_and_others table set preloaded once) and VectorE
    (iterative reciprocal) to balance the two queues.
"""

import numpy as np
import ml_dtypes

try:
    import concourse.bass as bass
    import concourse.bacc as bacc
    import concourse.tile as tile
    from concourse import mybir
    from concourse.bass_utils import run_bass_kernel_spmd
except ImportError:  # pragma: no cover
    import sys

    sys.path.insert(0, "/opt/trn_rl_repo")
    import concourse.bass as bass
    import concourse.bacc as bacc
    import concourse.tile as tile
    from concourse import mybir
    from concourse.bass_utils import run_bass_kernel_spmd

from contextlib import ExitStack

BF = ml_dtypes.bfloat16
N_CORES = 8
NPAIR = 2
DH = 64
HQ = WQ = 48
HK = WK = 24
NQ = HQ * WQ  # 2304

# s(i) = clip(i//2 - 4, 0, 15)
_S = np.clip(np.arange(48) // 2 - 4, 0, 15)

# merged bands: m covers s in {2m, 2m+1}; key rows 2m..2m+9; query rows:
MB = []  # (m, h0, nrows)
for m in range(8):
    hs = [h for h in range(48) if _S[h] in (2 * m, 2 * m + 1)]
    MB.append((m, hs[0], len(hs)))

# units: groups of (mb, h-sub-range) sharing one PSUM bank.
# middle mbs (r=4): two mbs per unit; big mbs (r=12): two h-halves (r=6).
# entry: list of (mb index, h0, r) sub-blocks.
UNITS = [
    [(1, MB[1][1], 4), (2, MB[2][1], 4)],
    [(3, MB[3][1], 4), (4, MB[4][1], 4)],
    [(5, MB[5][1], 4), (6, MB[6][1], 4)],
    [(0, 0, 6)],
    [(0, 6, 6)],
    [(7, 36, 6)],
    [(7, 42, 6)],
]


def build_kernel(ctx: ExitStack, tc, qb, ks, vs, out):
    nc = tc.nc
    FP32 = mybir.dt.float32
    BF16 = mybir.dt.bfloat16
    Exp = mybir.ActivationFunctionType.Exp
    Ln = mybir.ActivationFunctionType.Ln

    # Preload the ACT table set holding BOTH exp and ln, else the compiler's
    # per-activation set picker thrashes (~2.6us per ln-based reciprocal).
    from concourse.hw_specs import get_activation_tables

    set_id = list(get_activation_tables(nc.m.arch)).index("natural_log_exp_and_others")
    nc.scalar.add_instruction(
        mybir.InstLoadActFuncSet(
            name=nc.get_next_instruction_name(), act_func_set_id=set_id
        )
    )

    qpool = ctx.enter_context(tc.tile_pool(name="qt", bufs=1))
    kpool = ctx.enter_context(tc.tile_pool(name="kt", bufs=1))
    vpool = ctx.enter_context(tc.tile_pool(name="vt", bufs=1))
    spool = ctx.enter_context(tc.tile_pool(name="sc", bufs=2, space="PSUM"))
    opool = ctx.enter_context(tc.tile_pool(name="pv", bufs=2, space="PSUM"))
    mpool = ctx.enter_context(tc.tile_pool(name="sm", bufs=2, space="PSUM"))
    epool = ctx.enter_context(tc.tile_pool(name="ex", bufs=3))
    rpool = ctx.enter_context(tc.tile_pool(name="rc", bufs=2))
    respool = ctx.enter_context(tc.tile_pool(name="res", bufs=1))

    ones = vpool.tile([128, 64], BF16, tag="ones", name="ones")
    nc.vector.memset(ones[:, :], 1.0)

    qts, kts, vts = [], [], []
    for p in range(NPAIR):
        vt = vpool.tile([128, 1024], BF16, tag=f"vt{p}", name=f"vt{p}")
        nc.gpsimd.dma_start(vt[:, :], vs[:, 1024 * p : 1024 * p + 1024])
        vts.append(vt)
    for p in range(NPAIR):
        qt = qpool.tile([96, NQ], BF16, tag=f"qt{p}", name=f"qt{p}")
        nc.sync.dma_start(qt[:, :], qb[96 * p : 96 * p + 96, :])
        kt = kpool.tile([96, 2048], BF16, tag=f"kt{p}", name=f"kt{p}")
        nc.sync.dma_start(kt[:, :], ks[:, 2048 * p : 2048 * p + 2048])
        qts.append(qt)
        kts.append(kt)

    res = respool.tile([128, NQ], FP32, tag="res", name="res")
    # res viewed as [128, mbh(12), wb(3), hx(64)]: col = wb*768 + mbh*64 + hx
    resv = res.rearrange("p (wb mbh hx) -> p mbh wb hx", wb=3, mbh=12, hx=64)

    for ui, unit in enumerate(UNITS):
        # ---- scores + exp, per pair ----
        es = []
        ucols = sum(4 * 16 * r for (_, _, r) in unit)  # score cols in unit
        for p in range(NPAIR):
            qt, kt = qts[p], kts[p]
            qv = qt.rearrange("p (wb rest) -> p wb rest", wb=3)
            sc = spool.tile([128, 512], FP32, tag="sc")
            off = 0
            for (m, h0, r) in unit:
                bw = 16 * r
                for ci in range(2):  # chunk A, B
                    j = p * 16 + m * 2 + ci
                    rhs = qv[:, ci : ci + 2, h0 * 16 : h0 * 16 + bw]
                    nc.tensor.matmul(
                        sc[:, off : off + 2 * bw],
                        kt[:, 128 * (m * 2 + ci) : 128 * (m * 2 + ci) + 128],
                        rhs,
                        start=True,
                        stop=True,
                    )
                    off += 2 * bw
            e = epool.tile([128, 512], BF16, tag="e")
            nc.scalar.activation(e[:, :ucols], sc[:, :ucols], Exp)
            es.append(e)

        # ---- PV + sums matmuls, pair-stacked ----
        ocols = sum(3 * 16 * r for (_, _, r) in unit)
        pv = opool.tile([128, 512], FP32, tag="pv")
        sm = mpool.tile([128, 512], FP32, tag="sm")
        for p in range(NPAIR):
            e, vt = es[p], vts[p]
            soff = 0  # e-tile col offset
            ooff = 0  # out col offset
            for (m, h0, r) in unit:
                bw = 16 * r
                for ci in range(2):
                    dst0 = ooff + ci * bw  # A: wb0|wb1, B: wb1|wb2
                    nc.tensor.matmul(
                        pv[64 * p : 64 * p + 64, dst0 : dst0 + 2 * bw],
                        vt[:, 64 * (m * 2 + ci) : 64 * (m * 2 + ci) + 64],
                        e[:, soff : soff + 2 * bw],
                        start=(ci == 0),
                        stop=(ci == 1),
                    )
                    nc.tensor.matmul(
                        sm[64 * p : 64 * p + 64, dst0 : dst0 + 2 * bw],
                        ones[:, :],
                        e[:, soff : soff + 2 * bw],
                        start=(ci == 0),
                        stop=(ci == 1),
                    )
                    soff += 2 * bw
                ooff += 3 * bw

        # ---- normalization: rcp on ScalarE (exp(-ln)) or DVE, then mul ----
        rc = rpool.tile([128, 512], FP32, tag="rc")
        if ui < 3:  # the three 384-col middle units -> ScalarE
            lnm = rpool.tile([128, 512], FP32, tag="lnm")
            nc.scalar.activation(lnm[:, :ocols], sm[:, :ocols], Ln)
            nc.scalar.activation(rc[:, :ocols], lnm[:, :ocols], Exp, scale=-1.0)
        else:  # the four 288-col big-half units -> DVE
            nc.vector.reciprocal(rc[:, :ocols], sm[:, :ocols])

        # strided write into the persistent res tile
        if len(unit) == 2:
            (m, h0, r) = unit[0]
            mbh0 = h0 // 4
            dst = resv[:, mbh0 : mbh0 + 2, :, :]
            src_pv = pv[:, :ocols].rearrange(
                "p (mb wb hx) -> p mb wb hx", mb=2, wb=3, hx=64
            )
            src_rc = rc[:, :ocols].rearrange(
                "p (mb wb hx) -> p mb wb hx", mb=2, wb=3, hx=64
            )
        else:
            (m, h0, r) = unit[0]
            rv = res.rearrange("p (wb rest) -> p wb rest", wb=3)
            dst = rv[:, :, h0 * 16 : h0 * 16 + 96]
            src_pv = pv[:, :ocols].rearrange("p (wb hx) -> p wb hx", wb=3, hx=96)
            src_rc = rc[:, :ocols].rearrange("p (wb hx) -> p wb hx", wb=3, hx=96)
        nc.vector.tensor_mul(dst, src_pv, src_rc)

    nc.sync.dma_start(out[:, :], res[:, :])


_CACHE = {}


def _get_nc():
    if "nc" not in _CACHE:
        nc = bacc.Bacc(
            "TRN2", target_bir_lowering=False, debug=False, num_devices=N_CORES
        )
        qb = nc.dram_tensor(
            "qb", [NPAIR * 96, NQ], mybir.dt.bfloat16, kind="ExternalInput"
        ).ap()
        ks = nc.dram_tensor(
            "ks", [96, NPAIR * 2048], mybir.dt.bfloat16, kind="ExternalInput"
        ).ap()
        vs = nc.dram_tensor(
            "vs", [128, NPAIR * 1024], mybir.dt.bfloat16, kind="ExternalInput"
        ).ap()
        out = nc.dram_tensor(
            "out", [128, NQ], mybir.dt.float32, kind="ExternalOutput"
        ).ap()
        with tile.TileContext(nc) as tc, ExitStack() as ctx:
            build_kernel(ctx, tc, qb, ks, vs, out)
        nc.compile()
        _CACHE["nc"] = nc
    return _CACHE["nc"]


def _wb_blocked(a):
    """[C, 48, 48] -> [C, 2304] with col = wb*768 + h*16 + wi."""
    C = a.shape[0]
    return a.reshape(C, 48, 3, 16).transpose(0, 2, 1, 3).reshape(C, NQ)


def kernel(q: np.ndarray, k: np.ndarray, v: np.ndarray) -> np.ndarray:
    assert q.shape == (2, 512, HQ, WQ) and k.shape == (2, 512, HK, WK)
    nc = _get_nc()

    # mask tables
    m0 = np.full((16, 24), -30.0, np.float32)
    m1 = np.full((16, 24), -30.0, np.float32)
    for r in range(16):
        m0[r, r : r + 9] = 0.0
        m1[r, r : r + 9] = 0.0

    in_maps = []
    for c in range(N_CORES):
        qbc = np.zeros((NPAIR * 96, NQ), BF)
        ksc = np.zeros((96, NPAIR * 2048), np.float32)
        vsc = np.zeros((128, NPAIR * 1024), BF)
        for pl in range(NPAIR):
            pg = NPAIR * c + pl
            b, hd = pg // 8, pg % 8
            q4 = q[b, 64 * hd : 64 * hd + 64].reshape(64, 48, 48) / 8.0
            qq = np.zeros((96, 48, 48), np.float32)
            qq[0:64] = q4
            for w in range(48):
                qq[64 + _S[w], :, w] = 1.0
            for h in range(48):
                qq[80 + _S[h], h, :] = 1.0
            qbc[96 * pl : 96 * pl + 96] = _wb_blocked(qq).astype(BF)

            k4 = k[b, 64 * hd : 64 * hd + 64].reshape(64, 24, 24)
            v4 = v[b, 64 * hd : 64 * hd + 64].reshape(64, 24, 24)
            for m in range(8):
                for ci, c0 in ((0, 0), (1, 12)):
                    j = 16 * pl + 2 * m + ci
                    ksc[0:64, 128 * j : 128 * j + 120] = k4[
                        :, 2 * m : 2 * m + 10, c0 : c0 + 12
                    ].reshape(64, 120)
                    ksc[64:80, 128 * j : 128 * j + 120] = np.tile(
                        m0[:, c0 : c0 + 12], (1, 10)
                    )
                    ksc[80:96, 128 * j : 128 * j + 120] = np.repeat(
                        m1[:, 2 * m : 2 * m + 10], 12, axis=1
                    )
                    ksc[64:96, 128 * j + 120 : 128 * j + 128] = -30.0
                    vsc[0:120, 64 * j : 64 * j + 64] = (
                        v4[:, 2 * m : 2 * m + 10, c0 : c0 + 12].reshape(64, 120).T
                    )
        in_maps.append({"qb": qbc, "ks": ksc.astype(BF), "vs": vsc})

    results = run_bass_kernel_spmd(nc, in_maps, list(range(N_CORES))).results

    out = np.empty((2, 512, HQ, WQ), np.float32)
    for c in range(N_CORES):
        o = results[c]["out"]
        for pl in range(NPAIR):
            pg = NPAIR * c + pl
            b, hd = pg // 8, pg % 8
            out[b, 64 * hd : 64 * hd + 64] = (
                o[64 * pl : 64 * pl + 64]
                .reshape(64, 3, 48, 16)
                .transpose(0, 2, 1, 3)
                .reshape(64, 48, 48)
            )
    return out


if __name__ == "__main__":
    qq = np.load("/root/problem/q.npy")
    kk = np.load("/root/problem/k.npy")
    vv = np.load("/root/problem/v.npy")
    got = kernel(qq, kk, vv)
    exp = np.load("/root/problem/expected.npy")
    rel = np.linalg.norm(got - exp) / np.linalg.norm(exp)
    print("Relative error:", rel)


# revision 14
# speedup vs baseline: 1.8355x; 1.0507x over previous
"""Trainium2 Bass kernel for nn_CrossAttention_82471962018390.

Dilated (d=2) 9x9 neighborhood cross-attention, q 48x48 vs k/v 24x24.

Math identity: the nearest-exact 2x upsample + dilation-2 NATTEN window
collapses so query (h, w) attends the ORIGINAL 24x24 k/v grid at rows
clip(h//2-4,0,15)+0..8, cols clip(w//2-4,0,15)+0..8.

Structure (per core: 2 (b, head) pairs; 8 cores = 16 pairs):
  - 8 MERGED BANDS m = band pair (2m, 2m+1): key rows 2m..2m+9 (10 rows).
    Row-window membership folded into the matmul with 16 extra contraction
    rows (one-hot of s_h x M1 row-mask), on top of the 16 col-mask rows.
    Contraction = 64 dh + 16 colmask + 16 rowmask = 96.
  - Each merged band splits keys by column into overlapping chunks
    A = cols 0..11, B = cols 12..23 (120 keys each, padded to 128 with
    fully-masked keys).  Queries w0..15 need only A, w32..47 only B,
    w16..31 both -> 1.33 avg chunks/query instead of 2.
  - Query columns are laid out w-block-major: col = wb*768 + h*16 + wi,
    so each (chunk, wb-range) streams as one strided matmul rhs.
  - exp on ScalarE PSUM->SBUF bf16.
  - PV and sumexp(via ones stationary) matmuls are PAIR-STACKED: pair0 in
    PSUM partitions 0:64, pair1 in 64:128, so the softmax normalization
    (reciprocal + multiply) runs on all 128 DVE lanes, halving its column
    count.  Chunk A writes (wb0|wb1), chunk B (wb1|wb2) accumulating via
    PSUM per-element has_written bits.
  - 1/sumexp split between ScalarE (exp(-ln x), both fns in the
    natural_log_exp_and_others table set preloaded once) and VectorE
    (iterative reciprocal) to balance the two queues.
"""

import numpy as np
import ml_dtypes

try:
    import concourse.bass as bass
    import concourse.bacc as bacc
    import concourse.tile as tile
    from concourse import mybir
    from concourse.bass_utils import run_bass_kernel_spmd
except ImportError:  # pragma: no cover
    import sys

    sys.path.insert(0, "/opt/trn_rl_repo")
    import concourse.bass as bass
    import concourse.bacc as bacc
    import concourse.tile as tile
    from concourse import mybir
    from concourse.bass_utils import run_bass_kernel_spmd

from contextlib import ExitStack

BF = ml_dtypes.bfloat16
N_CORES = 8
NPAIR = 2
DH = 64
HQ = WQ = 48
HK = WK = 24
NQ = HQ * WQ  # 2304

# s(i) = clip(i//2 - 4, 0, 15)
_S = np.clip(np.arange(48) // 2 - 4, 0, 15)

# merged bands: m covers s in {2m, 2m+1}; key rows 2m..2m+9; query rows:
MB = []  # (m, h0, nrows)
for m in range(8):
    hs = [h for h in range(48) if _S[h] in (2 * m, 2 * m + 1)]
    MB.append((m, hs[0], len(hs)))

# units: groups of (mb, h-sub-range) sharing one PSUM bank.
# middle mbs (r=4): two mbs per unit; big mbs (r=12): two h-halves (r=6).
# entry: list of (mb index, h0, r) sub-blocks.
UNITS = [
    [(1, MB[1][1], 4), (2, MB[2][1], 4)],
    [(3, MB[3][1], 4), (4, MB[4][1], 4)],
    [(5, MB[5][1], 4), (6, MB[6][1], 4)],
    [(0, 0, 6)],
    [(0, 6, 6)],
    [(7, 36, 6)],
    [(7, 42, 6)],
]


def build_kernel(ctx: ExitStack, tc, qb, ks, vs, out):
    nc = tc.nc
    FP32 = mybir.dt.float32
    BF16 = mybir.dt.bfloat16
    Exp = mybir.ActivationFunctionType.Exp
    Ln = mybir.ActivationFunctionType.Ln

    # Preload the ACT table set holding BOTH exp and ln, else the compiler's
    # per-activation set picker thrashes (~2.6us per ln-based reciprocal).
    from concourse.hw_specs import get_activation_tables

    set_id = list(get_activation_tables(nc.m.arch)).index("natural_log_exp_and_others")
    nc.scalar.add_instruction(
        mybir.InstLoadActFuncSet(
            name=nc.get_next_instruction_name(), act_func_set_id=set_id
        )
    )

    qpool = ctx.enter_context(tc.tile_pool(name="qt", bufs=1))
    kpool = ctx.enter_context(tc.tile_pool(name="kt", bufs=1))
    vpool = ctx.enter_context(tc.tile_pool(name="vt", bufs=1))
    spool = ctx.enter_context(tc.tile_pool(name="sc", bufs=2, space="PSUM"))
    opool = ctx.enter_context(tc.tile_pool(name="pv", bufs=2, space="PSUM"))
    mpool = ctx.enter_context(tc.tile_pool(name="sm", bufs=2, space="PSUM"))
    epool = ctx.enter_context(tc.tile_pool(name="ex", bufs=3))
    rpool = ctx.enter_context(tc.tile_pool(name="rc", bufs=2))
    respool = ctx.enter_context(tc.tile_pool(name="res", bufs=1))

    ones = vpool.tile([128, 64], BF16, tag="ones", name="ones")
    nc.vector.memset(ones[:, :], 1.0)

    # Input DMAs spread across engine queues (each queue is its own DGE ring,
    # so the transfers run in parallel), ordered so unit 0's operands land
    # first: pair-0 k-slabs for mbs 1-2 (cols 256:768) ahead of the rest.
    vts = []
    for p in range(NPAIR):
        vt = vpool.tile([128, 1024], BF16, tag=f"vt{p}", name=f"vt{p}")
        vts.append(vt)
    kt0 = kpool.tile([96, 2048], BF16, tag="kt0", name="kt0")
    kt1 = kpool.tile([96, 2048], BF16, tag="kt1", name="kt1")
    qt0 = qpool.tile([96, NQ], BF16, tag="qt0", name="qt0")
    qt1 = qpool.tile([96, NQ], BF16, tag="qt1", name="qt1")
    # sync ring: pair-0 K slabs (unit 0 first), then pair-1 K
    nc.sync.dma_start(kt0[:, 256:768], ks[:, 256:768])
    nc.sync.dma_start(kt0[:, 0:256], ks[:, 0:256])
    nc.sync.dma_start(kt0[:, 768:2048], ks[:, 768:2048])
    nc.sync.dma_start(kt1[:, :], ks[:, 2048:4096])
    # scalar ring: queries
    nc.scalar.dma_start(qt0[:, 0:1536], qb[0:96, 0:1536])
    nc.scalar.dma_start(qt0[:, 1536:2304], qb[0:96, 1536:2304])
    nc.scalar.dma_start(qt1[:, :], qb[96:192, :])
    # gpsimd ring: V slabs
    nc.gpsimd.dma_start(vts[0][:, :], vs[:, 0:1024])
    nc.gpsimd.dma_start(vts[1][:, :], vs[:, 1024:2048])
    qts = [qt0, qt1]
    kts = [kt0, kt1]

    # HAM warm-up: ~4us of back-to-back dummy matmuls while the input DMAs
    # are in flight, so the PE clock is ungated (1.2 -> 2.4 GHz) before the
    # real matmuls begin.  Zero operands, private PSUM tile, no deps.
    wsb = vpool.tile([128, 512], BF16, tag="wsb", name="wsb")
    nc.gpsimd.memset(wsb[:, :], 0.0)
    wps = spool.tile([128, 512], FP32, tag="warm")
    for _ in range(20):
        nc.tensor.matmul(wps[:, :], wsb[:, 0:128], wsb[:, :], start=True, stop=True)

    res = respool.tile([128, NQ], FP32, tag="res", name="res")
    # res viewed as [128, mbh(12), wb(3), hx(64)]: col = wb*768 + mbh*64 + hx
    resv = res.rearrange("p (wb mbh hx) -> p mbh wb hx", wb=3, mbh=12, hx=64)

    for ui, unit in enumerate(UNITS):
        # ---- scores + exp, per pair ----
        es = []
        ucols = sum(4 * 16 * r for (_, _, r) in unit)  # score cols in unit
        for p in range(NPAIR):
            qt, kt = qts[p], kts[p]
            qv = qt.rearrange("p (wb rest) -> p wb rest", wb=3)
            sc = spool.tile([128, 512], FP32, tag="sc")
            off = 0
            for (m, h0, r) in unit:
                bw = 16 * r
                for ci in range(2):  # chunk A, B
                    j = p * 16 + m * 2 + ci
                    rhs = qv[:, ci : ci + 2, h0 * 16 : h0 * 16 + bw]
                    nc.tensor.matmul(
                        sc[:, off : off + 2 * bw],
                        kt[:, 128 * (m * 2 + ci) : 128 * (m * 2 + ci) + 128],
                        rhs,
                        start=True,
                        stop=True,
                    )
                    off += 2 * bw
            e = epool.tile([128, 512], BF16, tag="e")
            nc.scalar.activation(e[:, :ucols], sc[:, :ucols], Exp)
            es.append(e)

        # ---- PV + sums matmuls, pair-stacked ----
        ocols = sum(3 * 16 * r for (_, _, r) in unit)
        pv = opool.tile([128, 512], FP32, tag="pv")
        sm = mpool.tile([128, 512], FP32, tag="sm")
        for p in range(NPAIR):
            e, vt = es[p], vts[p]
            soff = 0  # e-tile col offset
            ooff = 0  # out col offset
            for (m, h0, r) in unit:
                bw = 16 * r
                for ci in range(2):
                    dst0 = ooff + ci * bw  # A: wb0|wb1, B: wb1|wb2
                    nc.tensor.matmul(
                        pv[64 * p : 64 * p + 64, dst0 : dst0 + 2 * bw],
                        vt[:, 64 * (m * 2 + ci) : 64 * (m * 2 + ci) + 64],
                        e[:, soff : soff + 2 * bw],
                        start=(ci == 0),
                        stop=(ci == 1),
                    )
                    nc.tensor.matmul(
                        sm[64 * p : 64 * p + 64, dst0 : dst0 + 2 * bw],
                        ones[:, :],
                        e[:, soff : soff + 2 * bw],
                        start=(ci == 0),
                        stop=(ci == 1),
                    )
                    soff += 2 * bw
                ooff += 3 * bw

        # ---- normalization: rcp on ScalarE (exp(-ln)) or DVE, then mul ----
        rc = rpool.tile([128, 512], FP32, tag="rc")
        if ui < 3:  # the three 384-col middle units -> ScalarE
            lnm = rpool.tile([128, 512], FP32, tag="lnm")
            nc.scalar.activation(lnm[:, :ocols], sm[:, :ocols], Ln)
            nc.scalar.activation(rc[:, :ocols], lnm[:, :ocols], Exp, scale=-1.0)
        else:  # the four 288-col big-half units -> DVE
            nc.vector.reciprocal(rc[:, :ocols], sm[:, :ocols])

        # strided write into the persistent res tile
        if len(unit) == 2:
            (m, h0, r) = unit[0]
            mbh0 = h0 // 4
            dst = resv[:, mbh0 : mbh0 + 2, :, :]
            src_pv = pv[:, :ocols].rearrange(
                "p (mb wb hx) -> p mb wb hx", mb=2, wb=3, hx=64
            )
            src_rc = rc[:, :ocols].rearrange(
                "p (mb wb hx) -> p mb wb hx", mb=2, wb=3, hx=64
            )
        else:
            (m, h0, r) = unit[0]
            rv = res.rearrange("p (wb rest) -> p wb rest", wb=3)
            dst = rv[:, :, h0 * 16 : h0 * 16 + 96]
            src_pv = pv[:, :ocols].rearrange("p (wb hx) -> p wb hx", wb=3, hx=96)
            src_rc = rc[:, :ocols].rearrange("p (wb hx) -> p wb hx", wb=3, hx=96)
        nc.vector.tensor_mul(dst, src_pv, src_rc)

    nc.sync.dma_start(out[:, :], res[:, :])


_CACHE = {}


def _get_nc():
    if "nc" not in _CACHE:
        nc = bacc.Bacc(
            "TRN2", target_bir_lowering=False, debug=False, num_devices=N_CORES
        )
        qb = nc.dram_tensor(
            "qb", [NPAIR * 96, NQ], mybir.dt.bfloat16, kind="ExternalInput"
        ).ap()
        ks = nc.dram_tensor(
            "ks", [96, NPAIR * 2048], mybir.dt.bfloat16, kind="ExternalInput"
        ).ap()
        vs = nc.dram_tensor(
            "vs", [128, NPAIR * 1024], mybir.dt.bfloat16, kind="ExternalInput"
        ).ap()
        out = nc.dram_tensor(
            "out", [128, NQ], mybir.dt.float32, kind="ExternalOutput"
        ).ap()
        with tile.TileContext(nc) as tc, ExitStack() as ctx:
            build_kernel(ctx, tc, qb, ks, vs, out)
        nc.compile()
        _CACHE["nc"] = nc
    return _CACHE["nc"]


def _wb_blocked(a):
    """[C, 48, 48] -> [C, 2304] with col = wb*768 + h*16 + wi."""
    C = a.shape[0]
    return a.reshape(C, 48, 3, 16).transpose(0, 2, 1, 3).reshape(C, NQ)


def kernel(q: np.ndarray, k: np.ndarray, v: np.ndarray) -> np.ndarray:
    assert q.shape == (2, 512, HQ, WQ) and k.shape == (2, 512, HK, WK)
    nc = _get_nc()

    # mask tables
    m0 = np.full((16, 24), -30.0, np.float32)
    m1 = np.full((16, 24), -30.0, np.float32)
    for r in range(16):
        m0[r, r : r + 9] = 0.0
        m1[r, r : r + 9] = 0.0

    in_maps = []
    for c in range(N_CORES):
        qbc = np.zeros((NPAIR * 96, NQ), BF)
        ksc = np.zeros((96, NPAIR * 2048), np.float32)
        vsc = np.zeros((128, NPAIR * 1024), BF)
        for pl in range(NPAIR):
            pg = NPAIR * c + pl
            b, hd = pg // 8, pg % 8
            q4 = q[b, 64 * hd : 64 * hd + 64].reshape(64, 48, 48) / 8.0
            qq = np.zeros((96, 48, 48), np.float32)
            qq[0:64] = q4
            for w in range(48):
                qq[64 + _S[w], :, w] = 1.0
            for h in range(48):
                qq[80 + _S[h], h, :] = 1.0
            qbc[96 * pl : 96 * pl + 96] = _wb_blocked(qq).astype(BF)

            k4 = k[b, 64 * hd : 64 * hd + 64].reshape(64, 24, 24)
            v4 = v[b, 64 * hd : 64 * hd + 64].reshape(64, 24, 24)
            for m in range(8):
                for ci, c0 in ((0, 0), (1, 12)):
                    j = 16 * pl + 2 * m + ci
                    ksc[0:64, 128 * j : 128 * j + 120] = k4[
                        :, 2 * m : 2 * m + 10, c0 : c0 + 12
                    ].reshape(64, 120)
                    ksc[64:80, 128 * j : 128 * j + 120] = np.tile(
                        m0[:, c0 : c0 + 12], (1, 10)
                    )
                    ksc[80:96, 128 * j : 128 * j + 120] = np.repeat(
                        m1[:, 2 * m : 2 * m + 10], 12, axis=1
                    )
                    ksc[64:96, 128 * j + 120 : 128 * j + 128] = -30.0
                    vsc[0:120, 64 * j : 64 * j + 64] = (
                        v4[:, 2 * m : 2 * m + 10, c0 : c0 + 12].reshape(64, 120).T
                    )
        in_maps.append({"qb": qbc, "ks": ksc.astype(BF), "vs": vsc})

    results = run_bass_kernel_spmd(nc, in_maps, list(range(N_CORES))).results

    out = np.empty((2, 512, HQ, WQ), np.float32)
    for c in range(N_CORES):
        o = results[c]["out"]
        for pl in range(NPAIR):
            pg = NPAIR * c + pl
            b, hd = pg // 8, pg % 8
            out[b, 64 * hd : 64 * hd + 64] = (
                o[64 * pl : 64 * pl + 64]
                .reshape(64, 3, 48, 16)
                .transpose(0, 2, 1, 3)
                .reshape(64, 48, 48)
            )
    return out


if __name__ == "__main__":
    qq = np.load("/root/problem/q.npy")
    kk = np.load("/root/problem/k.npy")
    vv = np.load("/root/problem/v.npy")
    got = kernel(qq, kk, vv)
    exp = np.load("/root/problem/expected.npy")
    rel = np.linalg.norm(got - exp) / np.linalg.norm(exp)
    print("Relative error:", rel)


# revision 18
# speedup vs baseline: 1.9013x; 1.0358x over previous
"""Trainium2 Bass kernel for nn_CrossAttention_82471962018390.

Dilated (d=2) 9x9 neighborhood cross-attention, q 48x48 vs k/v 24x24.

Math identity: the nearest-exact 2x upsample + dilation-2 NATTEN window
collapses so query (h, w) attends the ORIGINAL 24x24 k/v grid at rows
clip(h//2-4,0,15)+0..8, cols clip(w//2-4,0,15)+0..8.

Structure (per core: 2 (b, head) pairs; 8 cores = 16 pairs):
  - 8 MERGED BANDS m = band pair (2m, 2m+1): key rows 2m..2m+9 (10 rows).
    Row-window membership folded into the matmul with 16 extra contraction
    rows (one-hot of s_h x M1 row-mask), on top of the 16 col-mask rows.
    Contraction = 64 dh + 16 colmask + 16 rowmask = 96.
  - Each merged band splits keys by column into overlapping chunks
    A = cols 0..11, B = cols 12..23 (120 keys each, padded to 128 with
    fully-masked keys).  Queries w0..15 need only A, w32..47 only B,
    w16..31 both -> 1.33 avg chunks/query instead of 2.
  - Query columns are laid out w-block-major: col = wb*768 + h*16 + wi,
    so each (chunk, wb-range) streams as one strided matmul rhs.
  - exp on ScalarE PSUM->SBUF bf16.
  - PV and sumexp(via ones stationary) matmuls are PAIR-STACKED: pair0 in
    PSUM partitions 0:64, pair1 in 64:128, so the softmax normalization
    (reciprocal + multiply) runs on all 128 DVE lanes, halving its column
    count.  Chunk A writes (wb0|wb1), chunk B (wb1|wb2) accumulating via
    PSUM per-element has_written bits.
  - 1/sumexp split between ScalarE (exp(-ln x), both fns in the
    natural_log_exp_and_others table set preloaded once) and VectorE
    (iterative reciprocal) to balance the two queues.
"""

import numpy as np
import ml_dtypes

try:
    import concourse.bass as bass
    import concourse.bacc as bacc
    import concourse.tile as tile
    from concourse import mybir
    from concourse.bass_utils import run_bass_kernel_spmd
except ImportError:  # pragma: no cover
    import sys

    sys.path.insert(0, "/opt/trn_rl_repo")
    import concourse.bass as bass
    import concourse.bacc as bacc
    import concourse.tile as tile
    from concourse import mybir
    from concourse.bass_utils import run_bass_kernel_spmd

from contextlib import ExitStack

BF = ml_dtypes.bfloat16
N_CORES = 8
NPAIR = 2
DH = 64
HQ = WQ = 48
HK = WK = 24
NQ = HQ * WQ  # 2304

# s(i) = clip(i//2 - 4, 0, 15)
_S = np.clip(np.arange(48) // 2 - 4, 0, 15)

# merged bands: m covers s in {2m, 2m+1}; key rows 2m..2m+9; query rows:
MB = []  # (m, h0, nrows)
for m in range(8):
    hs = [h for h in range(48) if _S[h] in (2 * m, 2 * m + 1)]
    MB.append((m, hs[0], len(hs)))

# units: groups of (mb, h-sub-range) sharing one PSUM bank.
# middle mbs (r=4): two mbs per unit; big mbs (r=12): two h-halves (r=6).
# entry: list of (mb index, h0, r) sub-blocks.
UNITS = [
    [(1, MB[1][1], 4), (2, MB[2][1], 4)],
    [(3, MB[3][1], 4), (4, MB[4][1], 4)],
    [(5, MB[5][1], 4), (6, MB[6][1], 4)],
    [(0, 0, 6)],
    [(0, 6, 6)],
    [(7, 36, 6)],
    [(7, 42, 6)],
]


def build_kernel(ctx: ExitStack, tc, qb, ks, vs, out):
    nc = tc.nc
    FP32 = mybir.dt.float32
    BF16 = mybir.dt.bfloat16
    Exp = mybir.ActivationFunctionType.Exp
    Ln = mybir.ActivationFunctionType.Ln

    # Preload the ACT table set holding BOTH exp and ln, else the compiler's
    # per-activation set picker thrashes (~2.6us per ln-based reciprocal).
    from concourse.hw_specs import get_activation_tables

    set_id = list(get_activation_tables(nc.m.arch)).index("natural_log_exp_and_others")
    nc.scalar.add_instruction(
        mybir.InstLoadActFuncSet(
            name=nc.get_next_instruction_name(), act_func_set_id=set_id
        )
    )

    qpool = ctx.enter_context(tc.tile_pool(name="qt", bufs=1))
    kpool = ctx.enter_context(tc.tile_pool(name="kt", bufs=1))
    vpool = ctx.enter_context(tc.tile_pool(name="vt", bufs=1))
    spool = ctx.enter_context(tc.tile_pool(name="sc", bufs=2, space="PSUM"))
    opool = ctx.enter_context(tc.tile_pool(name="pv", bufs=2, space="PSUM"))
    mpool = ctx.enter_context(tc.tile_pool(name="sm", bufs=2, space="PSUM"))
    epool = ctx.enter_context(tc.tile_pool(name="ex", bufs=3))
    rpool = ctx.enter_context(tc.tile_pool(name="rc", bufs=2))
    respool = ctx.enter_context(tc.tile_pool(name="res", bufs=1))

    ones = vpool.tile([128, 64], BF16, tag="ones", name="ones")
    nc.vector.memset(ones[:, :], 1.0)
    # HAM warm-up operand: memset FIRST on the gpsimd queue (before its DMAs)
    # so the dummy matmuls can begin as soon as the engines come up.
    wsb = vpool.tile([128, 512], BF16, tag="wsb", name="wsb")
    nc.gpsimd.memset(wsb[:, :], 0.0)

    # Input DMAs spread across engine queues (each queue is its own DGE ring,
    # so the transfers run in parallel), ordered so unit 0's operands land
    # first: pair-0 k-slabs for mbs 1-2 (cols 256:768) ahead of the rest.
    vts = []
    for p in range(NPAIR):
        vt = vpool.tile([128, 1024], BF16, tag=f"vt{p}", name=f"vt{p}")
        vts.append(vt)
    kt0 = kpool.tile([96, 2048], BF16, tag="kt0", name="kt0")
    kt1 = kpool.tile([96, 2048], BF16, tag="kt1", name="kt1")
    qt0 = qpool.tile([96, NQ], BF16, tag="qt0", name="qt0")
    qt1 = qpool.tile([96, NQ], BF16, tag="qt1", name="qt1")
    # sync ring: pair-0 K slabs (unit 0 first), then pair-1 K
    nc.sync.dma_start(kt0[:, 256:768], ks[:, 256:768])
    nc.sync.dma_start(kt0[:, 0:256], ks[:, 0:256])
    nc.sync.dma_start(kt0[:, 768:2048], ks[:, 768:2048])
    nc.sync.dma_start(kt1[:, :], ks[:, 2048:4096])
    # scalar ring: queries
    nc.scalar.dma_start(qt0[:, 0:1536], qb[0:96, 0:1536])
    nc.scalar.dma_start(qt0[:, 1536:2304], qb[0:96, 1536:2304])
    nc.scalar.dma_start(qt1[:, :], qb[96:192, :])
    # gpsimd ring: V slabs
    nc.gpsimd.dma_start(vts[0][:, :], vs[:, 0:1024])
    nc.gpsimd.dma_start(vts[1][:, :], vs[:, 1024:2048])
    qts = [qt0, qt1]
    kts = [kt0, kt1]

    # HAM warm-up: ~4us of back-to-back dummy matmuls while the input DMAs
    # are in flight, so the PE clock is ungated (1.2 -> 2.4 GHz) before the
    # real matmuls begin.  Zero operands, private PSUM tile, no deps.
    wps = spool.tile([128, 512], FP32, tag="warm")
    for _ in range(20):
        nc.tensor.matmul(wps[:, :], wsb[:, 0:128], wsb[:, :], start=True, stop=True)

    res = respool.tile([128, NQ], BF16, tag="res", name="res")
    # res viewed as [128, mbh(12), wb(3), hx(64)]: col = wb*768 + mbh*64 + hx
    resv = res.rearrange("p (wb mbh hx) -> p mbh wb hx", wb=3, mbh=12, hx=64)

    for ui, unit in enumerate(UNITS):
        # ---- scores + exp, per pair ----
        es = []
        ucols = sum(4 * 16 * r for (_, _, r) in unit)  # score cols in unit
        for p in range(NPAIR):
            qt, kt = qts[p], kts[p]
            qv = qt.rearrange("p (wb rest) -> p wb rest", wb=3)
            sc = spool.tile([128, 512], FP32, tag="sc")
            off = 0
            for (m, h0, r) in unit:
                bw = 16 * r
                for ci in range(2):  # chunk A, B
                    j = p * 16 + m * 2 + ci
                    rhs = qv[:, ci : ci + 2, h0 * 16 : h0 * 16 + bw]
                    nc.tensor.matmul(
                        sc[:, off : off + 2 * bw],
                        kt[:, 128 * (m * 2 + ci) : 128 * (m * 2 + ci) + 128],
                        rhs,
                        start=True,
                        stop=True,
                    )
                    off += 2 * bw
            e = epool.tile([128, 512], BF16, tag="e")
            nc.scalar.activation(e[:, :ucols], sc[:, :ucols], Exp)
            es.append(e)

        # ---- PV + sums matmuls, pair-stacked ----
        ocols = sum(3 * 16 * r for (_, _, r) in unit)
        pv = opool.tile([128, 512], FP32, tag="pv")
        sm = mpool.tile([128, 512], FP32, tag="sm")
        for p in range(NPAIR):
            e, vt = es[p], vts[p]
            soff = 0  # e-tile col offset
            ooff = 0  # out col offset
            for (m, h0, r) in unit:
                bw = 16 * r
                for ci in range(2):
                    dst0 = ooff + ci * bw  # A: wb0|wb1, B: wb1|wb2
                    nc.tensor.matmul(
                        pv[64 * p : 64 * p + 64, dst0 : dst0 + 2 * bw],
                        vt[:, 64 * (m * 2 + ci) : 64 * (m * 2 + ci) + 64],
                        e[:, soff : soff + 2 * bw],
                        start=(ci == 0),
                        stop=(ci == 1),
                    )
                    nc.tensor.matmul(
                        sm[64 * p : 64 * p + 64, dst0 : dst0 + 2 * bw],
                        ones[:, :],
                        e[:, soff : soff + 2 * bw],
                        start=(ci == 0),
                        stop=(ci == 1),
                    )
                    soff += 2 * bw
                ooff += 3 * bw

        # ---- normalization: rcp on ScalarE (exp(-ln)) or DVE, then mul ----
        rc = rpool.tile([128, 512], FP32, tag="rc")
        if ui in (1, 2, 6):  # ScalarE recips, incl the LAST unit so the
            # tail's normalization overlaps DVE finishing unit 5
            lnm = rpool.tile([128, 512], FP32, tag="lnm")
            nc.scalar.activation(lnm[:, :ocols], sm[:, :ocols], Ln)
            nc.scalar.activation(rc[:, :ocols], lnm[:, :ocols], Exp, scale=-1.0)
        else:  # the four 288-col big-half units -> DVE
            nc.vector.reciprocal(rc[:, :ocols], sm[:, :ocols])

        # strided write into the persistent res tile
        if len(unit) == 2:
            (m, h0, r) = unit[0]
            mbh0 = h0 // 4
            dst = resv[:, mbh0 : mbh0 + 2, :, :]
            src_pv = pv[:, :ocols].rearrange(
                "p (mb wb hx) -> p mb wb hx", mb=2, wb=3, hx=64
            )
            src_rc = rc[:, :ocols].rearrange(
                "p (mb wb hx) -> p mb wb hx", mb=2, wb=3, hx=64
            )
        else:
            (m, h0, r) = unit[0]
            rv = res.rearrange("p (wb rest) -> p wb rest", wb=3)
            dst = rv[:, :, h0 * 16 : h0 * 16 + 96]
            src_pv = pv[:, :ocols].rearrange("p (wb hx) -> p wb hx", wb=3, hx=96)
            src_rc = rc[:, :ocols].rearrange("p (wb hx) -> p wb hx", wb=3, hx=96)
        nc.vector.tensor_mul(dst, src_pv, src_rc)

    nc.sync.dma_start(out[:, :], res[:, :])


_CACHE = {}


def _get_nc():
    if "nc" not in _CACHE:
        nc = bacc.Bacc(
            "TRN2", target_bir_lowering=False, debug=False, num_devices=N_CORES
        )
        qb = nc.dram_tensor(
            "qb", [NPAIR * 96, NQ], mybir.dt.bfloat16, kind="ExternalInput"
        ).ap()
        ks = nc.dram_tensor(
            "ks", [96, NPAIR * 2048], mybir.dt.bfloat16, kind="ExternalInput"
        ).ap()
        vs = nc.dram_tensor(
            "vs", [128, NPAIR * 1024], mybir.dt.bfloat16, kind="ExternalInput"
        ).ap()
        out = nc.dram_tensor(
            "out", [128, NQ], mybir.dt.bfloat16, kind="ExternalOutput"
        ).ap()
        with tile.TileContext(nc) as tc, ExitStack() as ctx:
            build_kernel(ctx, tc, qb, ks, vs, out)
        nc.compile()
        _CACHE["nc"] = nc
    return _CACHE["nc"]


def _wb_blocked(a):
    """[C, 48, 48] -> [C, 2304] with col = wb*768 + h*16 + wi."""
    C = a.shape[0]
    return a.reshape(C, 48, 3, 16).transpose(0, 2, 1, 3).reshape(C, NQ)


def kernel(q: np.ndarray, k: np.ndarray, v: np.ndarray) -> np.ndarray:
    assert q.shape == (2, 512, HQ, WQ) and k.shape == (2, 512, HK, WK)
    nc = _get_nc()

    # mask tables
    m0 = np.full((16, 24), -30.0, np.float32)
    m1 = np.full((16, 24), -30.0, np.float32)
    for r in range(16):
        m0[r, r : r + 9] = 0.0
        m1[r, r : r + 9] = 0.0

    in_maps = []
    for c in range(N_CORES):
        qbc = np.zeros((NPAIR * 96, NQ), BF)
        ksc = np.zeros((96, NPAIR * 2048), np.float32)
        vsc = np.zeros((128, NPAIR * 1024), BF)
        for pl in range(NPAIR):
            pg = NPAIR * c + pl
            b, hd = pg // 8, pg % 8
            q4 = q[b, 64 * hd : 64 * hd + 64].reshape(64, 48, 48) / 8.0
            qq = np.zeros((96, 48, 48), np.float32)
            qq[0:64] = q4
            for w in range(48):
                qq[64 + _S[w], :, w] = 1.0
            for h in range(48):
                qq[80 + _S[h], h, :] = 1.0
            qbc[96 * pl : 96 * pl + 96] = _wb_blocked(qq).astype(BF)

            k4 = k[b, 64 * hd : 64 * hd + 64].reshape(64, 24, 24)
            v4 = v[b, 64 * hd : 64 * hd + 64].reshape(64, 24, 24)
            for m in range(8):
                for ci, c0 in ((0, 0), (1, 12)):
                    j = 16 * pl + 2 * m + ci
                    ksc[0:64, 128 * j : 128 * j + 120] = k4[
                        :, 2 * m : 2 * m + 10, c0 : c0 + 12
                    ].reshape(64, 120)
                    ksc[64:80, 128 * j : 128 * j + 120] = np.tile(
                        m0[:, c0 : c0 + 12], (1, 10)
                    )
                    ksc[80:96, 128 * j : 128 * j + 120] = np.repeat(
                        m1[:, 2 * m : 2 * m + 10], 12, axis=1
                    )
                    ksc[64:96, 128 * j + 120 : 128 * j + 128] = -30.0
                    vsc[0:120, 64 * j : 64 * j + 64] = (
                        v4[:, 2 * m : 2 * m + 10, c0 : c0 + 12].reshape(64, 120).T
                    )
        in_maps.append({"qb": qbc, "ks": ksc.astype(BF), "vs": vsc})

    results = run_bass_kernel_spmd(nc, in_maps, list(range(N_CORES))).results

    out = np.empty((2, 512, HQ, WQ), np.float32)
    for c in range(N_CORES):
        o = results[c]["out"]
        for pl in range(NPAIR):
            pg = NPAIR * c + pl
            b, hd = pg // 8, pg % 8
            out[b, 64 * hd : 64 * hd + 64] = (
                o[64 * pl : 64 * pl + 64]
                .astype(np.float32)
                .reshape(64, 3, 48, 16)
                .transpose(0, 2, 1, 3)
                .reshape(64, 48, 48)
            )
    return out


if __name__ == "__main__":
    qq = np.load("/root/problem/q.npy")
    kk = np.load("/root/problem/k.npy")
    vv = np.load("/root/problem/v.npy")
    got = kernel(qq, kk, vv)
    exp = np.load("/root/problem/expected.npy")
    rel = np.linalg.norm(got - exp) / np.linalg.norm(exp)
    print("Relative error:", rel)
